# revision 2
# baseline (speedup 1.0000x reference)
"""Trainium2 Bass kernel for nn_HeteroForecastSageConv.

Strategy (8 NeuronCores, SPMD):
 - Destination-shard the 100000 target nodes across 8 cores (12800/core,
   padded to 102400). Each core computes the full pretransform
   x_t = relu(x_target @ Wp_t + bp_t) (duplicated, bf16) and writes it
   node-major to its HBM, then gathers source rows for its shard's edges
   with dma_gather (int16 indices -> 4 source-range bins of 25600 rows),
   segment-sums them via one-hot matmuls on the tensor engine
   (dst-major PSUM accumulation), applies mean via per-partition scale,
   and runs the folded epilogue matmuls.
 - Per-core inputs are rotated by the shard offset so the same program
   runs on every core (SPMD): core k receives x_target^T rolled by
   -12800k columns; gather indices are pre-rotated on the host.
 - Edge lists are preprocessed on the host into per-(core, direction,
   block, bin) budget-padded index/dst-local arrays; all segment
   bookkeeping is static in the program.

Math (alpha = 0.5, folded on host):
  x_mid = x_t @ (0.5 W_self + 0.5 W_ct_r + I) + aggS @ (0.25 W_s2d)
        + aggD @ (0.25 W_d2s) + aggC @ (0.5 W_ct_l) + b_mid
  out   = relu(x_mid) @ W_out + b_out
  b_mid = 0.5 b_self + 0.25 b_s2d + 0.25 b_d2s + 0.5 b_ct_l
"""
import sys
import dataclasses

sys.path.insert(0, "/opt/trn_rl_repo")

import numpy as np
import ml_dtypes

import concourse.bass as bass
import concourse.bacc as bacc
import concourse.mybir as mybir
import concourse.tile as tile
from concourse.tile import add_dep_helper
from concourse import bass_utils

BF16 = ml_dtypes.bfloat16
F32 = np.float32
NCORE = 8
P = 128


@dataclasses.dataclass(frozen=True)
class Cfg:
    n_t: int      # real target nodes
    n_c: int      # real context nodes
    shard: int    # target nodes per core (multiple of 128)
    nc_pad: int   # padded context nodes (multiple of stw)
    nbin: int     # source bins for tt gathers
    grp: int      # blocks per phase-B group
    stw: int      # phase-A super-tile width (multiple of 256)

    @property
    def nt_pad(self):
        return self.shard * NCORE

    @property
    def nblk(self):
        return self.shard // P

    @property
    def binsz(self):
        return self.nt_pad // self.nbin


FULL = Cfg(n_t=100000, n_c=20000, shard=12800, nc_pad=20480, nbin=4, grp=4, stw=4096)

_prog_cache = {}
USE_EXPLICIT_DEPS = False


def _groups(cfg):
    return [(g0, min(cfg.grp, cfg.nblk - g0)) for g0 in range(0, cfg.nblk, cfg.grp)]


def _wrap_idx(stream):
    """dma_gather index layout: idx j -> [j%16, j//16], tiled to 128 partitions."""
    assert stream.size % 16 == 0
    idx16 = stream.reshape(-1, 16).T
    return np.ascontiguousarray(np.tile(idx16, (8, 1)).astype(np.int16))


def build_program(cfg: Cfg, B_tt: int, B_ct: int):
    CC = B_tt // P
    CCc = B_ct // P
    nblk, nbin, grp = cfg.nblk, cfg.nbin, cfg.grp
    dt = mybir.dt

    nc = bacc.Bacc("TRN2", target_bir_lowering=False, debug=False)

    def din(name, shape, d):
        return nc.dram_tensor(name, shape, d, kind="ExternalInput")

    t_xT = din("xT", [P, cfg.nt_pad], dt.bfloat16)
    t_xcT = din("xcT", [P, cfg.nc_pad], dt.bfloat16)
    t_wpt = din("wpt", [P, P], dt.bfloat16)
    t_wpc = din("wpc", [P, P], dt.bfloat16)
    t_bpt = din("bpt", [P, 1], dt.float32)
    t_bpc = din("bpc", [P, 1], dt.float32)
    t_w1 = din("w1", [P, P], dt.bfloat16)
    t_ws = din("ws", [P, P], dt.bfloat16)
    t_wd = din("wd", [P, P], dt.bfloat16)
    t_wc = din("wc", [P, P], dt.bfloat16)
    t_wo = din("wo", [P, P], dt.bfloat16)
    t_bmid = din("bmid", [P, 1], dt.float32)
    t_bout = din("bout", [P, 1], dt.float32)
    t_iota = din("iota", [P, P], dt.bfloat16)
    t_ident = din("ident", [P, P], dt.bfloat16)
    L_tt = 2 * nblk * B_tt       # per-bin call stream length (both tt dirs)
    L_ct = nblk * B_ct
    t_idx = [din(f"idx{b}", [P, L_tt // 16], dt.int16) for b in range(nbin)]
    t_idxc = din("idxc", [P, L_ct // 16], dt.int16)
    t_dls = din("dls", [P, nblk * nbin * CC], dt.bfloat16)
    t_dld = din("dld", [P, nblk * nbin * CC], dt.bfloat16)
    t_dlc = din("dlc", [P, nblk * CCc], dt.bfloat16)
    t_invs = din("invs", [P, nblk], dt.float32)
    t_invd = din("invd", [P, nblk], dt.float32)
    t_invc = din("invc", [P, nblk], dt.float32)
    t_out = nc.dram_tensor("outT", [P, cfg.shard], dt.bfloat16, kind="ExternalOutput")

    AF = mybir.ActivationFunctionType
    OP = mybir.AluOpType

    with tile.TileContext(nc) as tc:
        with tc.tile_pool(name="dram", bufs=1, space="DRAM") as dpool, \
             tc.tile_pool(name="persist", bufs=1) as pp:
            xtn = dpool.tile([cfg.nt_pad, P], dt.bfloat16)
            xcn = dpool.tile([cfg.nc_pad, P], dt.bfloat16)

            def load(t, shape, d):
                s = pp.tile(shape, d, name=f"sb_{t.name}")
                nc.sync.dma_start(s[:], t.ap())
                return s

            sb_wpt = load(t_wpt, [P, P], dt.bfloat16)
            sb_wpc = load(t_wpc, [P, P], dt.bfloat16)
            sb_bpt = load(t_bpt, [P, 1], dt.float32)
            sb_bpc = load(t_bpc, [P, 1], dt.float32)
            sb_w1 = load(t_w1, [P, P], dt.bfloat16)
            sb_ws = load(t_ws, [P, P], dt.bfloat16)
            sb_wd = load(t_wd, [P, P], dt.bfloat16)
            sb_wc = load(t_wc, [P, P], dt.bfloat16)
            sb_wo = load(t_wo, [P, P], dt.bfloat16)
            sb_bmid = load(t_bmid, [P, 1], dt.float32)
            sb_bout = load(t_bout, [P, 1], dt.float32)
            sb_iota = load(t_iota, [P, P], dt.bfloat16)
            sb_ident = load(t_ident, [P, P], dt.bfloat16)
            sb_idx = [load(t_idx[b], [P, L_tt // 16], dt.int16) for b in range(nbin)]
            sb_idxc = load(t_idxc, [P, L_ct // 16], dt.int16)
            sb_dls = load(t_dls, [P, nblk * nbin * CC], dt.bfloat16)
            sb_dld = load(t_dld, [P, nblk * nbin * CC], dt.bfloat16)
            sb_dlc = load(t_dlc, [P, nblk * CCc], dt.bfloat16)
            sb_invs = load(t_invs, [P, nblk], dt.float32)
            sb_invd = load(t_invd, [P, nblk], dt.float32)
            sb_invc = load(t_invc, [P, nblk], dt.float32)
            xt_mine = pp.tile([P, cfg.shard], dt.bfloat16)

            node_writes = []

            # ---------------- Phase A: pretransform ----------------
            with tc.tile_pool(name="pa", bufs=2) as pa, \
                 tc.tile_pool(name="psA", bufs=2, space="PSUM") as psA:

                def pretransform(src_dram, n_cols, w_sb, b_sb, nodes_dram, keep_mine):
                    for st0 in range(0, n_cols, cfg.stw):
                        sb_in = pa.tile([P, cfg.stw], dt.bfloat16, name="a_in", tag="a_in")
                        nc.sync.dma_start(sb_in[:], src_dram.ap()[:, st0:st0 + cfg.stw])
                        sb_nodes = pa.tile([P, cfg.stw], dt.bfloat16, name="a_nodes", tag="a_nodes")
                        for j in range(cfg.stw // 256):
                            col = st0 + 256 * j
                            ps = psA.tile([P, 256], dt.float32, name="a_ps", tag="a_ps")
                            nc.tensor.matmul(ps[:], lhsT=w_sb[:],
                                             rhs=sb_in[:, 256 * j:256 * j + 256],
                                             start=True, stop=True)
                            if keep_mine and col < cfg.shard:
                                dest = xt_mine[:, col:col + 256]
                            else:
                                scratch = pa.tile([P, 256], dt.bfloat16, name="a_feat", tag="a_feat")
                                dest = scratch[:]
                            nc.scalar.activation(dest, ps[:], AF.Relu, bias=b_sb[:, 0:1])
                            for h in range(2):
                                pst = psA.tile([P, P], dt.bfloat16, name="a_tr", tag="a_tr")
                                nc.tensor.transpose(pst[:], dest[:, P * h:P * h + P], sb_ident[:])
                                nc.vector.tensor_copy(
                                    sb_nodes[:, 256 * j + P * h:256 * j + P * h + P], pst[:])
                        w = nc.sync.dma_start(
                            nodes_dram[st0:st0 + cfg.stw, :]
                            .rearrange("(g p) f -> p g f", p=P),
                            sb_nodes[:].rearrange("p (g f) -> p g f", f=P))
                        node_writes.append(w)

                pretransform(t_xcT, cfg.nc_pad, sb_wpc, sb_bpc, xcn, False)
                pretransform(t_xT, cfg.nt_pad, sb_wpt, sb_bpt, xtn, True)

            # ---------------- Phase B: gather + aggregate + epilogue ----------------
            groups = _groups(cfg)
            # static per-group offsets into the per-bin call streams
            off_tt, off_ct = [], []
            o1 = o2 = 0
            for (g0, gn) in groups:
                off_tt.append(o1)
                off_ct.append(o2)
                o1 += 2 * gn * B_tt
                o2 += gn * B_ct
            assert o1 == L_tt and o2 == L_ct

            first_tt_gather = [None]
            first_ct_gather = [None]

            with tc.tile_pool(name="pb", bufs=2) as pb, \
                 tc.tile_pool(name="psB", bufs=2, space="PSUM") as psB:
                for gi, (g0, gn) in enumerate(groups):
                    xg = []
                    for b in range(nbin):
                        n_i = 2 * gn * B_tt
                        xgb = pb.tile([P, n_i // P, P], dt.bfloat16,
                                      name=f"xg{b}", tag=f"xg{b}")
                        g_inst = nc.gpsimd.dma_gather(
                            out_ap=xgb[:],
                            in_ap=xtn[cfg.binsz * b:cfg.binsz * (b + 1), :],
                            idxs_ap=sb_idx[b][:, off_tt[gi] // 16:(off_tt[gi] + n_i) // 16],
                            num_idxs=n_i, num_idxs_reg=n_i,
                            elem_size=P, single_packet=False)
                        if first_tt_gather[0] is None:
                            first_tt_gather[0] = g_inst
                        xg.append(xgb)
                    n_c_i = gn * B_ct
                    xgc = pb.tile([P, n_c_i // P, P], dt.bfloat16, name="xgc", tag="xgc")
                    gc_inst = nc.gpsimd.dma_gather(
                        out_ap=xgc[:], in_ap=xcn[:],
                        idxs_ap=sb_idxc[:, off_ct[gi] // 16:(off_ct[gi] + n_c_i) // 16],
                        num_idxs=n_c_i, num_idxs_reg=n_c_i,
                        elem_size=P, single_packet=False)
                    if first_ct_gather[0] is None:
                        first_ct_gather[0] = gc_inst

                    sb_og = pb.tile([P, P * gn], dt.bfloat16, name="og", tag="og")
                    for b_loc in range(gn):
                        blk = g0 + b_loc
                        # one-hot tiles (batched is_equal per direction)
                        oh_s = pb.tile([P, nbin * CC, P], dt.bfloat16, name="oh_s", tag="oh_s")
                        nc.vector.tensor_tensor(
                            out=oh_s[:],
                            in0=sb_iota[:].unsqueeze(1).to_broadcast([P, nbin * CC, P]),
                            in1=sb_dls[:, blk * nbin * CC:(blk + 1) * nbin * CC]
                                .unsqueeze(2).to_broadcast([P, nbin * CC, P]),
                            op=OP.is_equal)
                        oh_d = pb.tile([P, nbin * CC, P], dt.bfloat16, name="oh_d", tag="oh_d")
                        nc.vector.tensor_tensor(
                            out=oh_d[:],
                            in0=sb_iota[:].unsqueeze(1).to_broadcast([P, nbin * CC, P]),
                            in1=sb_dld[:, blk * nbin * CC:(blk + 1) * nbin * CC]
                                .unsqueeze(2).to_broadcast([P, nbin * CC, P]),
                            op=OP.is_equal)
                        oh_c = pb.tile([P, CCc, P], dt.bfloat16, name="oh_c", tag="oh_c")
                        nc.vector.tensor_tensor(
                            out=oh_c[:],
                            in0=sb_iota[:].unsqueeze(1).to_broadcast([P, CCc, P]),
                            in1=sb_dlc[:, blk * CCc:(blk + 1) * CCc]
                                .unsqueeze(2).to_broadcast([P, CCc, P]),
                            op=OP.is_equal)

                        ps_agg = psB.tile([P, 384], dt.float32, name="agg", tag="agg")
                        # s2d into [:, 0:128]
                        n_mm = nbin * CC
                        k = 0
                        for b in range(nbin):
                            for j in range(CC):
                                nc.tensor.matmul(
                                    ps_agg[:, 0:P],
                                    lhsT=oh_s[:, b * CC + j, :],
                                    rhs=xg[b][:, b_loc * CC + j, :],
                                    start=(k == 0), stop=(k == n_mm - 1))
                                k += 1
                        # d2s into [:, 128:256]
                        k = 0
                        for b in range(nbin):
                            for j in range(CC):
                                nc.tensor.matmul(
                                    ps_agg[:, P:2 * P],
                                    lhsT=oh_d[:, b * CC + j, :],
                                    rhs=xg[b][:, (gn + b_loc) * CC + j, :],
                                    start=(k == 0), stop=(k == n_mm - 1))
                                k += 1
                        # ct into [:, 256:384]
                        for j in range(CCc):
                            nc.tensor.matmul(
                                ps_agg[:, 2 * P:3 * P],
                                lhsT=oh_c[:, j, :],
                                rhs=xgc[:, b_loc * CCc + j, :],
                                start=(j == 0), stop=(j == CCc - 1))

                        # mean scale + transpose to feature-major
                        aggT = {}
                        for (reg, invt, nm) in ((0, sb_invs, "S"), (1, sb_invd, "D"),
                                                (2, sb_invc, "C")):
                            sba = pb.tile([P, P], dt.bfloat16, name=f"agg{nm}", tag=f"agg{nm}")
                            nc.scalar.activation(sba[:], ps_agg[:, reg * P:(reg + 1) * P],
                                                 AF.Copy, scale=invt[:, blk:blk + 1])
                            pst = psB.tile([P, P], dt.bfloat16, name="btr", tag="btr")
                            nc.tensor.transpose(pst[:], sba[:], sb_ident[:])
                            sbt = pb.tile([P, P], dt.bfloat16, name=f"aggT{nm}", tag=f"aggT{nm}")
                            nc.scalar.copy(sbt[:], pst[:])
                            aggT[nm] = sbt

                        # epilogue
                        ps_mid = psB.tile([P, P], dt.float32, name="mid", tag="mid")
                        nc.tensor.matmul(ps_mid[:], lhsT=sb_w1[:],
                                         rhs=xt_mine[:, P * blk:P * blk + P],
                                         start=True, stop=False)
                        nc.tensor.matmul(ps_mid[:], lhsT=sb_ws[:], rhs=aggT["S"][:],
                                         start=False, stop=False)
                        nc.tensor.matmul(ps_mid[:], lhsT=sb_wd[:], rhs=aggT["D"][:],
                                         start=False, stop=False)
                        nc.tensor.matmul(ps_mid[:], lhsT=sb_wc[:], rhs=aggT["C"][:],
                                         start=False, stop=True)
                        sb_mid = pb.tile([P, P], dt.bfloat16, name="mid_sb", tag="mid_sb")
                        nc.scalar.activation(sb_mid[:], ps_mid[:], AF.Relu,
                                             bias=sb_bmid[:, 0:1])
                        ps_out = psB.tile([P, P], dt.float32, name="out_ps", tag="out_ps")
                        nc.tensor.matmul(ps_out[:], lhsT=sb_wo[:], rhs=sb_mid[:],
                                         start=True, stop=True)
                        nc.scalar.activation(sb_og[:, P * b_loc:P * b_loc + P], ps_out[:],
                                             AF.Identity, bias=sb_bout[:, 0:1])
                    nc.sync.dma_start(t_out.ap()[:, P * g0:P * (g0 + gn)],
                                      sb_og[:, :P * gn])

            # explicit phase barrier: gathers must not start before the tables
            # are fully written (belt and braces on top of Tile's dep tracking)
            if USE_EXPLICIT_DEPS:
                for w in node_writes:
                    if first_tt_gather[0] is not None:
                        add_dep_helper(w.ins, first_tt_gather[0].ins, True, "phaseA->ttgather")
                    if first_ct_gather[0] is not None:
                        add_dep_helper(w.ins, first_ct_gather[0].ins, True, "phaseA->ctgather")

    nc.compile()
    return nc


def preprocess(inputs, cfg: Cfg):
    xt = np.asarray(inputs["x_target"], F32)
    xc = np.asarray(inputs["x_context"], F32)
    ett = np.asarray(inputs["edge_tt"]).astype(np.int64)
    ecs = np.asarray(inputs["edge_ct_src"]).astype(np.int64)
    ecd = np.asarray(inputs["edge_ct_dst"]).astype(np.int64)

    xtT = np.zeros((P, cfg.nt_pad), BF16)
    xtT[:, :xt.shape[0]] = xt.T.astype(BF16)
    xcT = np.zeros((P, cfg.nc_pad), BF16)
    xcT[:, :xc.shape[0]] = xc.T.astype(BF16)

    # folded weights
    W_self = np.asarray(inputs["W_self"], F32)
    W_ct_r = np.asarray(inputs["W_ct_r"], F32)
    w1 = 0.5 * W_self + 0.5 * W_ct_r + np.eye(P, dtype=F32)
    ws = 0.25 * np.asarray(inputs["W_s2d"], F32)
    wd = 0.25 * np.asarray(inputs["W_d2s"], F32)
    wc = 0.5 * np.asarray(inputs["W_ct_l"], F32)
    wo = np.asarray(inputs["W_out"], F32)
    bmid = (0.5 * np.asarray(inputs["b_self"], F32)
            + 0.25 * np.asarray(inputs["b_s2d"], F32)
            + 0.25 * np.asarray(inputs["b_d2s"], F32)
            + 0.5 * np.asarray(inputs["b_ct_l"], F32))
    bout = np.asarray(inputs["b_out"], F32)

    shared = {
        "xcT": xcT,
        "wpt": np.ascontiguousarray(np.asarray(inputs["Wp_t"], F32).astype(BF16)),
        "wpc": np.ascontiguousarray(np.asarray(inputs["Wp_c"], F32).astype(BF16)),
        "bpt": np.asarray(inputs["bp_t"], F32).reshape(P, 1),
        "bpc": np.asarray(inputs["bp_c"], F32).reshape(P, 1),
        "w1": w1.astype(BF16), "ws": ws.astype(BF16), "wd": wd.astype(BF16),
        "wc": wc.astype(BF16), "wo": wo.astype(BF16),
        "bmid": bmid.reshape(P, 1), "bout": bout.reshape(P, 1),
        "iota": np.ascontiguousarray(
            np.broadcast_to(np.arange(P, dtype=F32), (P, P)).astype(BF16)),
        "ident": np.eye(P, dtype=F32).astype(BF16),
    }

    dirs = {
        "s": (ett[1], ett[0], True),
        "d": (ett[0], ett[1], True),
        "c": (ecd, ecs, False),
    }

    nblk, nbin = cfg.nblk, cfg.nbin
    prepped = {}
    cellmax_tt = cellmax_ct = 0
    for nm, (key, gnode, is_tt) in dirs.items():
        core = key // cfg.shard
        block = (key % cfg.shard) // P
        dloc = (key % P).astype(F32)
        if is_tt:
            rot = (gnode - core * cfg.shard) % cfg.nt_pad
            bin_ = rot // cfg.binsz
            loc = rot % cfg.binsz
            cell = (core * nblk + block) * nbin + bin_
            ncell = NCORE * nblk * nbin
        else:
            loc = gnode
            cell = core * nblk + block
            ncell = NCORE * nblk
        order = np.argsort(cell, kind="stable")
        cell_s = cell[order]
        counts = np.bincount(cell_s, minlength=ncell)
        starts = np.concatenate([[0], np.cumsum(counts)[:-1]])
        pos = np.arange(len(cell_s)) - starts[cell_s]
        mx = int(counts.max()) if counts.size else 0
        prepped[nm] = (order, cell_s, pos, loc, dloc, ncell)
        if is_tt:
            cellmax_tt = max(cellmax_tt, mx)
        else:
            cellmax_ct = max(cellmax_ct, mx)

    B_tt = max(P, -(-cellmax_tt // P) * P)
    B_ct = max(P, -(-cellmax_ct // P) * P)

    def fill(nm, B):
        order, cell_s, pos, loc, dloc, ncell = prepped[nm]
        m_idx = np.zeros(ncell * B, np.int16)
        m_dl = np.full(ncell * B, -1.0, F32)
        slot = cell_s * B + pos
        m_idx[slot] = loc[order].astype(np.int16)
        m_dl[slot] = dloc[order]
        return m_idx, m_dl

    mi_s, md_s = fill("s", B_tt)
    mi_d, md_d = fill("d", B_tt)
    mi_c, md_c = fill("c", B_ct)
    # shapes per core
    mi_s = mi_s.reshape(NCORE, nblk, nbin, B_tt)
    md_s = md_s.reshape(NCORE, nblk, nbin, B_tt)
    mi_d = mi_d.reshape(NCORE, nblk, nbin, B_tt)
    md_d = md_d.reshape(NCORE, nblk, nbin, B_tt)
    mi_c = mi_c.reshape(NCORE, nblk, B_ct)
    md_c = md_c.reshape(NCORE, nblk, B_ct)

    inv = {}
    for nm, (key, _, _) in dirs.items():
        cnt = np.bincount(key, minlength=cfg.nt_pad)
        inv[nm] = (1.0 / np.maximum(cnt, 1)).astype(F32)

    groups = _groups(cfg)
    in_maps = []
    for k in range(NCORE):
        m = dict(shared)
        m["xT"] = np.roll(xtT, -cfg.shard * k, axis=1)
        for b in range(cfg.nbin):
            segs = []
            for (g0, gn) in groups:
                segs.append(mi_s[k, g0:g0 + gn, b, :].ravel())
                segs.append(mi_d[k, g0:g0 + gn, b, :].ravel())
            m[f"idx{b}"] = _wrap_idx(np.concatenate(segs))
        m["idxc"] = _wrap_idx(mi_c[k].ravel())
        m["dls"] = np.ascontiguousarray(md_s[k].reshape(-1, P).T.astype(BF16))
        m["dld"] = np.ascontiguousarray(md_d[k].reshape(-1, P).T.astype(BF16))
        m["dlc"] = np.ascontiguousarray(md_c[k].reshape(-1, P).T.astype(BF16))
        for nm in ("s", "d", "c"):
            m[f"inv{nm}"] = np.ascontiguousarray(
                inv[nm][k * cfg.shard:(k + 1) * cfg.shard].reshape(nblk, P).T)
        in_maps.append(m)
    return in_maps, B_tt, B_ct


def run(inputs, cfg: Cfg, trace=False):
    in_maps, B_tt, B_ct = preprocess(inputs, cfg)
    key = (cfg, B_tt, B_ct)
    if key not in _prog_cache:
        _prog_cache[key] = build_program(cfg, B_tt, B_ct)
    nc = _prog_cache[key]
    res = bass_utils.run_bass_kernel_spmd(nc, in_maps, core_ids=list(range(NCORE)),
                                          trace=trace)
    outT = np.concatenate([res.results[k]["outT"] for k in range(NCORE)], axis=1)
    n_t = np.asarray(inputs["x_target"]).shape[0]
    out = outT[:, :n_t].T.astype(F32)
    return out, res


def kernel(**inputs) -> np.ndarray:
    out, _ = run(inputs, FULL, trace=False)
    return out



# revision 7
# speedup vs baseline: 1.3806x; 1.3806x over previous
"""Trainium2 Bass kernel for nn_HeteroForecastSageConv.

Strategy (8 NeuronCores, SPMD):
 - Destination-shard the 100000 target nodes across 8 cores (12800/core,
   padded to 102400). Each core computes the full pretransform
   x_t = relu(x_target @ Wp_t + bp_t) (duplicated, bf16) and writes it
   node-major to its HBM, then gathers source rows for its shard's edges
   with dma_gather (int16 indices -> 4 source-range bins of 25600 rows),
   segment-sums them via one-hot matmuls on the tensor engine
   (dst-major PSUM accumulation), applies mean via per-partition scale,
   and runs the folded epilogue matmuls.
 - Per-core inputs are rotated by the shard offset so the same program
   runs on every core (SPMD): core k receives x_target^T rolled by
   -12800k columns; gather indices are pre-rotated on the host.
 - Edge lists are preprocessed on the host into per-(core, direction,
   block, bin) budget-padded index/dst-local arrays; all segment
   bookkeeping is static in the program.

Math (alpha = 0.5, folded on host):
  x_mid = x_t @ (0.5 W_self + 0.5 W_ct_r + I) + aggS @ (0.25 W_s2d)
        + aggD @ (0.25 W_d2s) + aggC @ (0.5 W_ct_l) + b_mid
  out   = relu(x_mid) @ W_out + b_out
  b_mid = 0.5 b_self + 0.25 b_s2d + 0.25 b_d2s + 0.5 b_ct_l
"""
import sys
import dataclasses

sys.path.insert(0, "/opt/trn_rl_repo")

import numpy as np
import ml_dtypes

import concourse.bass as bass
import concourse.bacc as bacc
import concourse.mybir as mybir
import concourse.tile as tile
from concourse.tile import add_dep_helper
from concourse import bass_utils

BF16 = ml_dtypes.bfloat16
F32 = np.float32
NCORE = 8
P = 128


@dataclasses.dataclass(frozen=True)
class Cfg:
    n_t: int      # real target nodes
    n_c: int      # real context nodes
    shard: int    # target nodes per core (multiple of 128)
    nc_pad: int   # padded context nodes (multiple of stw)
    nbin: int     # source bins for tt gathers
    grp: int      # blocks per phase-B group
    stw: int      # phase-A super-tile width (multiple of 256)

    @property
    def nt_pad(self):
        return self.shard * NCORE

    @property
    def nblk(self):
        return self.shard // P

    @property
    def binsz(self):
        return self.nt_pad // self.nbin


FULL = Cfg(n_t=100000, n_c=20000, shard=12800, nc_pad=20480, nbin=4, grp=8, stw=4096)


def _balance_nodes(deg_sd, deg_c, n_pos, cap):
    """Assign target nodes to positions so that per-(128-node block) sums of
    deg_sd and deg_c are near-uniform. Returns new_pos[node] -> position.

    LPT-style: deal nodes in decreasing deg_sd round-robin over blocks, then
    repair blocks whose deg_c sum is high by swapping with low-c blocks.
    """
    n_nodes = deg_sd.shape[0]
    nblk = n_pos // P
    order = np.argsort(-(deg_sd.astype(np.int64) * 4096) - deg_c, kind="stable")
    blk_of = np.empty(n_nodes, np.int32)
    blk_of[order] = np.arange(n_nodes, dtype=np.int32) % nblk

    # c-repair: greedily swap high-c nodes out of c-overloaded blocks for
    # low-c nodes from underloaded blocks, keeping deg_sd nearly unchanged.
    csum = np.bincount(blk_of, weights=deg_c, minlength=nblk).astype(np.int64)
    nodes_by_blk = [list(np.where(blk_of == b)[0]) for b in range(nblk)]
    cap_c = int(np.ceil(deg_c.sum() / nblk)) + 4
    for _ in range(20000):
        bh = int(np.argmax(csum))
        if csum[bh] <= cap_c:
            break
        bl = int(np.argmin(csum))
        nh = np.array(nodes_by_blk[bh])
        nl = np.array(nodes_by_blk[bl])
        diff = deg_c[nh][:, None].astype(np.int64) - deg_c[nl][None, :]
        good = np.abs(deg_sd[nh][:, None].astype(np.int64)
                      - deg_sd[nl][None, :]) <= 1
        diff = np.where(good, diff, -10)
        i, j = np.unravel_index(np.argmax(diff), diff.shape)
        if diff[i, j] <= 0:
            break
        u, v = int(nh[i]), int(nl[j])
        blk_of[u], blk_of[v] = bl, bh
        nodes_by_blk[bh][i], nodes_by_blk[bl][j] = v, u
        csum[bh] -= diff[i, j]
        csum[bl] += diff[i, j]

    # positions: nodes sorted by block, sequential slots inside each block
    pos_order = np.argsort(blk_of, kind="stable")
    counts = np.bincount(blk_of, minlength=nblk)
    assert counts.max() <= cap
    starts = np.concatenate([[0], np.cumsum(counts)[:-1]])
    within = np.arange(n_nodes) - starts[blk_of[pos_order]]
    new_pos = np.empty(n_nodes, np.int64)
    new_pos[pos_order] = blk_of[pos_order].astype(np.int64) * P + within
    return new_pos

_prog_cache = {}
USE_EXPLICIT_DEPS = False


def _groups(cfg):
    return [(g0, min(cfg.grp, cfg.nblk - g0)) for g0 in range(0, cfg.nblk, cfg.grp)]


def _wrap_idx(stream):
    """dma_gather index layout: idx j -> [j%16, j//16], tiled to 128 partitions."""
    assert stream.size % 16 == 0
    idx16 = stream.reshape(-1, 16).T
    return np.ascontiguousarray(np.tile(idx16, (8, 1)).astype(np.int16))


def build_program(cfg: Cfg, B_tt: int, B_ct: int):
    CC = B_tt // P
    CCc = B_ct // P
    nblk, nbin, grp = cfg.nblk, cfg.nbin, cfg.grp
    dt = mybir.dt

    nc = bacc.Bacc("TRN2", target_bir_lowering=False, debug=False)

    def din(name, shape, d):
        return nc.dram_tensor(name, shape, d, kind="ExternalInput")

    t_xT = din("xT", [P, cfg.nt_pad], dt.bfloat16)
    t_xcT = din("xcT", [P, cfg.nc_pad], dt.bfloat16)
    t_wpt = din("wpt", [P, P], dt.bfloat16)
    t_wpc = din("wpc", [P, P], dt.bfloat16)
    t_bpt = din("bpt", [P, 1], dt.float32)
    t_bpc = din("bpc", [P, 1], dt.float32)
    t_w1 = din("w1", [P, P], dt.bfloat16)
    t_ws = din("ws", [P, P], dt.bfloat16)
    t_wd = din("wd", [P, P], dt.bfloat16)
    t_wc = din("wc", [P, P], dt.bfloat16)
    t_wo = din("wo", [P, P], dt.bfloat16)
    t_bmid = din("bmid", [P, 1], dt.float32)
    t_bout = din("bout", [P, 1], dt.float32)
    t_iota = din("iota", [P, P], dt.bfloat16)
    t_ident = din("ident", [P, P], dt.bfloat16)
    L_tt = 2 * nblk * B_tt       # per-bin call stream length (both tt dirs)
    L_ct = nblk * B_ct
    t_idx = [din(f"idx{b}", [P, L_tt // 16], dt.int16) for b in range(nbin)]
    t_idxc = din("idxc", [P, L_ct // 16], dt.int16)
    t_dls = din("dls", [P, nblk * nbin * CC], dt.bfloat16)
    t_dld = din("dld", [P, nblk * nbin * CC], dt.bfloat16)
    t_dlc = din("dlc", [P, nblk * CCc], dt.bfloat16)
    t_invs = din("invs", [P, nblk], dt.float32)
    t_invd = din("invd", [P, nblk], dt.float32)
    t_invc = din("invc", [P, nblk], dt.float32)
    t_out = nc.dram_tensor("outT", [P, cfg.shard], dt.bfloat16, kind="ExternalOutput")

    AF = mybir.ActivationFunctionType
    OP = mybir.AluOpType

    with tile.TileContext(nc) as tc:
        with tc.tile_pool(name="dram", bufs=1, space="DRAM") as dpool, \
             tc.tile_pool(name="persist", bufs=1) as pp:
            xtn = dpool.tile([cfg.nt_pad, P], dt.bfloat16)
            xcn = dpool.tile([cfg.nc_pad, P], dt.bfloat16)

            def load(t, shape, d):
                s = pp.tile(shape, d, name=f"sb_{t.name}")
                nc.sync.dma_start(s[:], t.ap())
                return s

            sb_wpt = load(t_wpt, [P, P], dt.bfloat16)
            sb_wpc = load(t_wpc, [P, P], dt.bfloat16)
            sb_bpt = load(t_bpt, [P, 1], dt.float32)
            sb_bpc = load(t_bpc, [P, 1], dt.float32)
            sb_w1 = load(t_w1, [P, P], dt.bfloat16)
            sb_ws = load(t_ws, [P, P], dt.bfloat16)
            sb_wd = load(t_wd, [P, P], dt.bfloat16)
            sb_wc = load(t_wc, [P, P], dt.bfloat16)
            sb_wo = load(t_wo, [P, P], dt.bfloat16)
            sb_bmid = load(t_bmid, [P, 1], dt.float32)
            sb_bout = load(t_bout, [P, 1], dt.float32)
            sb_iota = load(t_iota, [P, P], dt.bfloat16)
            sb_ident = load(t_ident, [P, P], dt.bfloat16)
            sb_idx = [load(t_idx[b], [P, L_tt // 16], dt.int16) for b in range(nbin)]
            sb_idxc = load(t_idxc, [P, L_ct // 16], dt.int16)
            sb_dls = load(t_dls, [P, nblk * nbin * CC], dt.bfloat16)
            sb_dld = load(t_dld, [P, nblk * nbin * CC], dt.bfloat16)
            sb_dlc = load(t_dlc, [P, nblk * CCc], dt.bfloat16)
            sb_invs = load(t_invs, [P, nblk], dt.float32)
            sb_invd = load(t_invd, [P, nblk], dt.float32)
            sb_invc = load(t_invc, [P, nblk], dt.float32)
            xt_mine = pp.tile([P, cfg.shard], dt.bfloat16)

            node_writes = []

            # ---------------- Phase A: pretransform ----------------
            with tc.tile_pool(name="pa", bufs=2) as pa, \
                 tc.tile_pool(name="psA", bufs=2, space="PSUM") as psA:

                def pretransform(src_dram, n_cols, w_sb, b_sb, nodes_dram, keep_mine):
                    for st0 in range(0, n_cols, cfg.stw):
                        sb_in = pa.tile([P, cfg.stw], dt.bfloat16, name="a_in", tag="a_in")
                        nc.sync.dma_start(sb_in[:], src_dram.ap()[:, st0:st0 + cfg.stw])
                        sb_nodes = pa.tile([P, cfg.stw], dt.bfloat16, name="a_nodes", tag="a_nodes")
                        for j in range(cfg.stw // 256):
                            col = st0 + 256 * j
                            ps = psA.tile([P, 256], dt.float32, name="a_ps", tag="a_ps")
                            nc.tensor.matmul(ps[:], lhsT=w_sb[:],
                                             rhs=sb_in[:, 256 * j:256 * j + 256],
                                             start=True, stop=True)
                            if keep_mine and col < cfg.shard:
                                dest = xt_mine[:, col:col + 256]
                            else:
                                scratch = pa.tile([P, 256], dt.bfloat16, name="a_feat", tag="a_feat")
                                dest = scratch[:]
                            nc.scalar.activation(dest, ps[:], AF.Relu, bias=b_sb[:, 0:1])
                            for h in range(2):
                                pst = psA.tile([P, P], dt.bfloat16, name="a_tr", tag="a_tr")
                                nc.tensor.transpose(pst[:], dest[:, P * h:P * h + P], sb_ident[:])
                                nc.vector.tensor_copy(
                                    sb_nodes[:, 256 * j + P * h:256 * j + P * h + P], pst[:])
                        w = nc.sync.dma_start(
                            nodes_dram[st0:st0 + cfg.stw, :]
                            .rearrange("(g p) f -> p g f", p=P),
                            sb_nodes[:].rearrange("p (g f) -> p g f", f=P))
                        node_writes.append(w)

                pretransform(t_xcT, cfg.nc_pad, sb_wpc, sb_bpc, xcn, False)
                pretransform(t_xT, cfg.nt_pad, sb_wpt, sb_bpt, xtn, True)

            # ---------------- Phase B: gather + aggregate + epilogue ----------------
            groups = _groups(cfg)
            # static per-group offsets into the per-bin call streams
            off_tt, off_ct = [], []
            o1 = o2 = 0
            for (g0, gn) in groups:
                off_tt.append(o1)
                off_ct.append(o2)
                o1 += 2 * gn * B_tt
                o2 += gn * B_ct
            assert o1 == L_tt and o2 == L_ct

            first_tt_gather = [None]
            first_ct_gather = [None]

            with tc.tile_pool(name="pb", bufs=2) as pb, \
                 tc.tile_pool(name="psB", bufs=2, space="PSUM") as psB:
                for gi, (g0, gn) in enumerate(groups):
                    xg = []
                    for b in range(nbin):
                        n_i = 2 * gn * B_tt
                        xgb = pb.tile([P, n_i // P, P], dt.bfloat16,
                                      name=f"xg{b}", tag=f"xg{b}")
                        g_inst = nc.gpsimd.dma_gather(
                            out_ap=xgb[:],
                            in_ap=xtn[cfg.binsz * b:cfg.binsz * (b + 1), :],
                            idxs_ap=sb_idx[b][:, off_tt[gi] // 16:(off_tt[gi] + n_i) // 16],
                            num_idxs=n_i, num_idxs_reg=n_i,
                            elem_size=P, single_packet=False)
                        if first_tt_gather[0] is None:
                            first_tt_gather[0] = g_inst
                        xg.append(xgb)
                    n_c_i = gn * B_ct
                    xgc = pb.tile([P, n_c_i // P, P], dt.bfloat16, name="xgc", tag="xgc")
                    gc_inst = nc.gpsimd.dma_gather(
                        out_ap=xgc[:], in_ap=xcn[:],
                        idxs_ap=sb_idxc[:, off_ct[gi] // 16:(off_ct[gi] + n_c_i) // 16],
                        num_idxs=n_c_i, num_idxs_reg=n_c_i,
                        elem_size=P, single_packet=False)
                    if first_ct_gather[0] is None:
                        first_ct_gather[0] = gc_inst

                    sb_og = pb.tile([P, P * gn], dt.bfloat16, name="og", tag="og")
                    for b_loc in range(gn):
                        blk = g0 + b_loc
                        # one-hot tiles (batched is_equal per direction)
                        oh_s = pb.tile([P, nbin * CC, P], dt.bfloat16, name="oh_s", tag="oh_s")
                        nc.vector.tensor_tensor(
                            out=oh_s[:],
                            in0=sb_iota[:].unsqueeze(1).to_broadcast([P, nbin * CC, P]),
                            in1=sb_dls[:, blk * nbin * CC:(blk + 1) * nbin * CC]
                                .unsqueeze(2).to_broadcast([P, nbin * CC, P]),
                            op=OP.is_equal)
                        oh_d = pb.tile([P, nbin * CC, P], dt.bfloat16, name="oh_d", tag="oh_d")
                        nc.vector.tensor_tensor(
                            out=oh_d[:],
                            in0=sb_iota[:].unsqueeze(1).to_broadcast([P, nbin * CC, P]),
                            in1=sb_dld[:, blk * nbin * CC:(blk + 1) * nbin * CC]
                                .unsqueeze(2).to_broadcast([P, nbin * CC, P]),
                            op=OP.is_equal)
                        oh_c = pb.tile([P, CCc, P], dt.bfloat16, name="oh_c", tag="oh_c")
                        nc.vector.tensor_tensor(
                            out=oh_c[:],
                            in0=sb_iota[:].unsqueeze(1).to_broadcast([P, CCc, P]),
                            in1=sb_dlc[:, blk * CCc:(blk + 1) * CCc]
                                .unsqueeze(2).to_broadcast([P, CCc, P]),
                            op=OP.is_equal)

                        ps_agg = psB.tile([P, 384], dt.float32, name="agg", tag="agg")
                        # s2d into [:, 0:128]
                        n_mm = nbin * CC
                        k = 0
                        for b in range(nbin):
                            for j in range(CC):
                                nc.tensor.matmul(
                                    ps_agg[:, 0:P],
                                    lhsT=oh_s[:, b * CC + j, :],
                                    rhs=xg[b][:, b_loc * CC + j, :],
                                    start=(k == 0), stop=(k == n_mm - 1))
                                k += 1
                        # d2s into [:, 128:256]
                        k = 0
                        for b in range(nbin):
                            for j in range(CC):
                                nc.tensor.matmul(
                                    ps_agg[:, P:2 * P],
                                    lhsT=oh_d[:, b * CC + j, :],
                                    rhs=xg[b][:, (gn + b_loc) * CC + j, :],
                                    start=(k == 0), stop=(k == n_mm - 1))
                                k += 1
                        # ct into [:, 256:384]
                        for j in range(CCc):
                            nc.tensor.matmul(
                                ps_agg[:, 2 * P:3 * P],
                                lhsT=oh_c[:, j, :],
                                rhs=xgc[:, b_loc * CCc + j, :],
                                start=(j == 0), stop=(j == CCc - 1))

                        # mean scale + transpose to feature-major
                        aggT = {}
                        for (reg, invt, nm) in ((0, sb_invs, "S"), (1, sb_invd, "D"),
                                                (2, sb_invc, "C")):
                            sba = pb.tile([P, P], dt.bfloat16, name=f"agg{nm}", tag=f"agg{nm}")
                            nc.scalar.activation(sba[:], ps_agg[:, reg * P:(reg + 1) * P],
                                                 AF.Copy, scale=invt[:, blk:blk + 1])
                            pst = psB.tile([P, P], dt.bfloat16, name="btr", tag="btr")
                            nc.tensor.transpose(pst[:], sba[:], sb_ident[:])
                            sbt = pb.tile([P, P], dt.bfloat16, name=f"aggT{nm}", tag=f"aggT{nm}")
                            nc.scalar.copy(sbt[:], pst[:])
                            aggT[nm] = sbt

                        # epilogue
                        ps_mid = psB.tile([P, P], dt.float32, name="mid", tag="mid")
                        nc.tensor.matmul(ps_mid[:], lhsT=sb_w1[:],
                                         rhs=xt_mine[:, P * blk:P * blk + P],
                                         start=True, stop=False)
                        nc.tensor.matmul(ps_mid[:], lhsT=sb_ws[:], rhs=aggT["S"][:],
                                         start=False, stop=False)
                        nc.tensor.matmul(ps_mid[:], lhsT=sb_wd[:], rhs=aggT["D"][:],
                                         start=False, stop=False)
                        nc.tensor.matmul(ps_mid[:], lhsT=sb_wc[:], rhs=aggT["C"][:],
                                         start=False, stop=True)
                        sb_mid = pb.tile([P, P], dt.bfloat16, name="mid_sb", tag="mid_sb")
                        nc.scalar.activation(sb_mid[:], ps_mid[:], AF.Relu,
                                             bias=sb_bmid[:, 0:1])
                        ps_out = psB.tile([P, P], dt.float32, name="out_ps", tag="out_ps")
                        nc.tensor.matmul(ps_out[:], lhsT=sb_wo[:], rhs=sb_mid[:],
                                         start=True, stop=True)
                        nc.scalar.activation(sb_og[:, P * b_loc:P * b_loc + P], ps_out[:],
                                             AF.Identity, bias=sb_bout[:, 0:1])
                    nc.sync.dma_start(t_out.ap()[:, P * g0:P * (g0 + gn)],
                                      sb_og[:, :P * gn])

            # explicit phase barrier: gathers must not start before the tables
            # are fully written (belt and braces on top of Tile's dep tracking)
            if USE_EXPLICIT_DEPS:
                for w in node_writes:
                    if first_tt_gather[0] is not None:
                        add_dep_helper(w.ins, first_tt_gather[0].ins, True, "phaseA->ttgather")
                    if first_ct_gather[0] is not None:
                        add_dep_helper(w.ins, first_ct_gather[0].ins, True, "phaseA->ctgather")

    nc.compile()
    return nc


def preprocess(inputs, cfg: Cfg):
    xt = np.asarray(inputs["x_target"], F32)
    xc = np.asarray(inputs["x_context"], F32)
    ett = np.asarray(inputs["edge_tt"]).astype(np.int64)
    ecs = np.asarray(inputs["edge_ct_src"]).astype(np.int64)
    ecd = np.asarray(inputs["edge_ct_dst"]).astype(np.int64)

    # renumber target nodes to balance per-block degree sums (shrinks the
    # data-dependent gather budgets B_tt / B_ct)
    n_t = xt.shape[0]
    deg_s = np.bincount(ett[1], minlength=n_t)
    deg_d = np.bincount(ett[0], minlength=n_t)
    deg_c = np.bincount(ecd, minlength=n_t)
    new_pos = _balance_nodes((deg_s + deg_d).astype(np.int64),
                             deg_c.astype(np.int64), cfg.nt_pad, P)
    ett = new_pos[ett]
    ecd = new_pos[ecd]

    xtT = np.zeros((P, cfg.nt_pad), BF16)
    xtT[:, new_pos] = xt.T.astype(BF16)
    xcT = np.zeros((P, cfg.nc_pad), BF16)
    xcT[:, :xc.shape[0]] = xc.T.astype(BF16)

    # folded weights
    W_self = np.asarray(inputs["W_self"], F32)
    W_ct_r = np.asarray(inputs["W_ct_r"], F32)
    w1 = 0.5 * W_self + 0.5 * W_ct_r + np.eye(P, dtype=F32)
    ws = 0.25 * np.asarray(inputs["W_s2d"], F32)
    wd = 0.25 * np.asarray(inputs["W_d2s"], F32)
    wc = 0.5 * np.asarray(inputs["W_ct_l"], F32)
    wo = np.asarray(inputs["W_out"], F32)
    bmid = (0.5 * np.asarray(inputs["b_self"], F32)
            + 0.25 * np.asarray(inputs["b_s2d"], F32)
            + 0.25 * np.asarray(inputs["b_d2s"], F32)
            + 0.5 * np.asarray(inputs["b_ct_l"], F32))
    bout = np.asarray(inputs["b_out"], F32)

    shared = {
        "xcT": xcT,
        "wpt": np.ascontiguousarray(np.asarray(inputs["Wp_t"], F32).astype(BF16)),
        "wpc": np.ascontiguousarray(np.asarray(inputs["Wp_c"], F32).astype(BF16)),
        "bpt": np.asarray(inputs["bp_t"], F32).reshape(P, 1),
        "bpc": np.asarray(inputs["bp_c"], F32).reshape(P, 1),
        "w1": w1.astype(BF16), "ws": ws.astype(BF16), "wd": wd.astype(BF16),
        "wc": wc.astype(BF16), "wo": wo.astype(BF16),
        "bmid": bmid.reshape(P, 1), "bout": bout.reshape(P, 1),
        "iota": np.ascontiguousarray(
            np.broadcast_to(np.arange(P, dtype=F32), (P, P)).astype(BF16)),
        "ident": np.eye(P, dtype=F32).astype(BF16),
    }

    dirs = {
        "s": (ett[1], ett[0], True),
        "d": (ett[0], ett[1], True),
        "c": (ecd, ecs, False),
    }

    nblk, nbin = cfg.nblk, cfg.nbin
    prepped = {}
    cellmax_tt = cellmax_ct = 0
    for nm, (key, gnode, is_tt) in dirs.items():
        core = key // cfg.shard
        block = (key % cfg.shard) // P
        dloc = (key % P).astype(F32)
        if is_tt:
            rot = (gnode - core * cfg.shard) % cfg.nt_pad
            bin_ = rot // cfg.binsz
            loc = rot % cfg.binsz
            cell = (core * nblk + block) * nbin + bin_
            ncell = NCORE * nblk * nbin
        else:
            loc = gnode
            cell = core * nblk + block
            ncell = NCORE * nblk
        order = np.argsort(cell, kind="stable")
        cell_s = cell[order]
        counts = np.bincount(cell_s, minlength=ncell)
        starts = np.concatenate([[0], np.cumsum(counts)[:-1]])
        pos = np.arange(len(cell_s)) - starts[cell_s]
        mx = int(counts.max()) if counts.size else 0
        prepped[nm] = (order, cell_s, pos, loc, dloc, ncell)
        if is_tt:
            cellmax_tt = max(cellmax_tt, mx)
        else:
            cellmax_ct = max(cellmax_ct, mx)

    B_tt = max(P, -(-cellmax_tt // P) * P)
    B_ct = max(P, -(-cellmax_ct // P) * P)

    def fill(nm, B):
        order, cell_s, pos, loc, dloc, ncell = prepped[nm]
        m_idx = np.zeros(ncell * B, np.int16)
        m_dl = np.full(ncell * B, -1.0, F32)
        slot = cell_s * B + pos
        m_idx[slot] = loc[order].astype(np.int16)
        m_dl[slot] = dloc[order]
        return m_idx, m_dl

    mi_s, md_s = fill("s", B_tt)
    mi_d, md_d = fill("d", B_tt)
    mi_c, md_c = fill("c", B_ct)
    # shapes per core
    mi_s = mi_s.reshape(NCORE, nblk, nbin, B_tt)
    md_s = md_s.reshape(NCORE, nblk, nbin, B_tt)
    mi_d = mi_d.reshape(NCORE, nblk, nbin, B_tt)
    md_d = md_d.reshape(NCORE, nblk, nbin, B_tt)
    mi_c = mi_c.reshape(NCORE, nblk, B_ct)
    md_c = md_c.reshape(NCORE, nblk, B_ct)

    inv = {}
    for nm, (key, _, _) in dirs.items():
        cnt = np.bincount(key, minlength=cfg.nt_pad)
        inv[nm] = (1.0 / np.maximum(cnt, 1)).astype(F32)

    groups = _groups(cfg)
    in_maps = []
    for k in range(NCORE):
        m = dict(shared)
        m["xT"] = np.roll(xtT, -cfg.shard * k, axis=1)
        for b in range(cfg.nbin):
            segs = []
            for (g0, gn) in groups:
                segs.append(mi_s[k, g0:g0 + gn, b, :].ravel())
                segs.append(mi_d[k, g0:g0 + gn, b, :].ravel())
            m[f"idx{b}"] = _wrap_idx(np.concatenate(segs))
        m["idxc"] = _wrap_idx(mi_c[k].ravel())
        m["dls"] = np.ascontiguousarray(md_s[k].reshape(-1, P).T.astype(BF16))
        m["dld"] = np.ascontiguousarray(md_d[k].reshape(-1, P).T.astype(BF16))
        m["dlc"] = np.ascontiguousarray(md_c[k].reshape(-1, P).T.astype(BF16))
        for nm in ("s", "d", "c"):
            m[f"inv{nm}"] = np.ascontiguousarray(
                inv[nm][k * cfg.shard:(k + 1) * cfg.shard].reshape(nblk, P).T)
        in_maps.append(m)
    return in_maps, B_tt, B_ct, new_pos


def run(inputs, cfg: Cfg, trace=False):
    in_maps, B_tt, B_ct, new_pos = preprocess(inputs, cfg)
    print(f"budgets: B_tt={B_tt} B_ct={B_ct}", flush=True)
    key = (cfg, B_tt, B_ct)
    if key not in _prog_cache:
        _prog_cache[key] = build_program(cfg, B_tt, B_ct)
    nc = _prog_cache[key]
    res = bass_utils.run_bass_kernel_spmd(nc, in_maps, core_ids=list(range(NCORE)),
                                          trace=trace)
    outT = np.concatenate([res.results[k]["outT"] for k in range(NCORE)], axis=1)
    out = outT[:, new_pos].T.astype(F32)
    return out, res


def kernel(**inputs) -> np.ndarray:
    out, _ = run(inputs, FULL, trace=False)
    return out

